# revision 1
# baseline (speedup 1.0000x reference)
"""Chamfer distance kernel for Trainium2 (8 NeuronCores, data-parallel over batch).

Input : x, y float32 [16, 4096, 3]
Output: scalar float32 = mean_b [ mean_n min_m ||x_bn - y_bm||^2
                                + mean_m min_n ||x_bn - y_bm||^2 ]

Strategy (per core = 2 batches):
  Host: cast points to bf16 (x~, y~), build K=7 augmented vectors so that
        ax . ay = 2 x~.y~ - |x~|^2 - |y~|^2 = -d(x~,y~)   exactly in fp32
        (|.|^2 terms split hi/lo into two bf16 entries to avoid rounding).
  PE   : per 128-x block, 8 matmuls [7,128]^T @ [7,512] -> PSUM fp32 = -d.
  ACT  : cast PSUM fp32 -> SBUF bf16 (S tiles).
  DVE  : dirA: tensor_tensor_reduce(max of the two halves, fused row-max)
               -> rowacc[:, block]    (= -min_m d per x point)
         dirB: colrun = max(colrun, S) running over blocks.
  GPSIMD: batch finalize: partition_all_reduce over rowacc sums (add) and
          colrun (max); DVE sums -> scalar accumulator.
"""
import sys

sys.path.insert(0, "/opt/trn_rl_repo")

import numpy as np
import ml_dtypes

import concourse.bacc as bacc
import concourse.bass as bass
import concourse.bass_isa as bass_isa
import concourse.tile as tile
from concourse import mybir
from concourse.alu_op_type import AluOpType
from concourse.bass_utils import run_bass_kernel_spmd

F32 = mybir.dt.float32
BF16 = mybir.dt.bfloat16
X = mybir.AxisListType.X
A = AluOpType

B, N, D3 = 16, 4096, 3
NCORES = 8
BPC = B // NCORES           # batches per core
RB = N // 128               # 32 row blocks of x per batch
KAUG = 7                    # augmented contraction dim
HALF = N // 2               # 2048
NEG_INF = -1.0e30

import os
USE_TTR = os.environ.get("K_TTR", "0") == "1"        # fused fold+rowmax (ucode op
                                                     # rejected by HW path - keep 0)
PAR_BF16 = os.environ.get("K_PARBF16", "1") == "1"   # bf16 partition_all_reduce


def _build_nc(repeat: int = 1):
    nc = bacc.Bacc("TRN2", target_bir_lowering=False, debug=False, num_devices=NCORES)
    # ax[k, b*N + n]: [2x0, 2x1, 2x2, -x2hi, -x2lo, 1, 1]
    # ay[k, b*N + m]: [y0, y1, y2, 1, 1, -y2hi, -y2lo]
    ax_d = nc.dram_tensor("ax", [KAUG, BPC * N], BF16, kind="ExternalInput").ap()
    ay_d = nc.dram_tensor("ay", [KAUG, BPC * N], BF16, kind="ExternalInput").ap()
    out_d = nc.dram_tensor("out", [1, 1], F32, kind="ExternalOutput").ap()

    with tile.TileContext(nc) as tc:
        import contextlib
        with contextlib.ExitStack() as ctx:
            const = ctx.enter_context(tc.tile_pool(name="const", bufs=1))
            acc = ctx.enter_context(tc.tile_pool(name="acc", bufs=1))
            sp = ctx.enter_context(tc.tile_pool(name="sp", bufs=4))
            ps = ctx.enter_context(tc.tile_pool(name="ps", bufs=2, space="PSUM"))

            ax_t = const.tile([KAUG, BPC * N], BF16, name="ax_t")
            ay_t = const.tile([KAUG, BPC * N], BF16, name="ay_t")
            nc.gpsimd.dma_start(ax_t[:], ax_d[:])
            nc.gpsimd.dma_start(ay_t[:], ay_d[:])

            colrun = acc.tile([128, N], BF16, name="colrun")
            rowacc = acc.tile([128, RB], F32, name="rowacc")
            f1 = acc.tile([128, HALF], BF16, name="f1")
            f2 = acc.tile([128, HALF // 2], BF16, name="f2")
            f3 = acc.tile([128, HALF // 4], BF16, name="f3")
            cred = acc.tile([128, N], BF16, name="cred")
            stot = acc.tile([1, 1], F32, name="stot")
            s_out = acc.tile([1, 1], F32, name="s_out")

            for it in range(BPC * repeat):
                b = it % BPC
                for r in range(RB):
                    s_halves = []
                    for h in range(2):
                        psum_t = ps.tile([128, HALF], F32, name="psum", tag="psum")
                        for c in range(4):
                            mcol = b * N + h * HALF + c * 512
                            nc.tensor.matmul(
                                psum_t[:, c * 512:(c + 1) * 512],
                                ax_t[:, b * N + r * 128: b * N + (r + 1) * 128],
                                ay_t[:, mcol: mcol + 512],
                                start=True, stop=True,
                            )
                        s_h = sp.tile([128, HALF], BF16, name="s_h", tag="S")
                        nc.scalar.copy(s_h[:], psum_t[:])
                        # dirB: running per-column max (over x points)
                        cslice = colrun[:, h * HALF:(h + 1) * HALF]
                        if r == 0:
                            nc.vector.tensor_copy(cslice, s_h[:])
                        else:
                            nc.vector.tensor_tensor(cslice, cslice, s_h[:], op=A.max)
                        s_halves.append(s_h)
                    # dirA: fused fold-of-halves + row max -> rowacc[:, r]
                    if USE_TTR:
                        nc.vector.tensor_tensor_reduce(
                            out=f1[:],
                            in0=s_halves[0][:],
                            in1=s_halves[1][:],
                            scale=1.0,
                            scalar=NEG_INF,
                            op0=A.max,
                            op1=A.max,
                            accum_out=rowacc[:, r: r + 1],
                        )
                    else:
                        nc.vector.tensor_tensor(f1[:], s_halves[0][:],
                                                s_halves[1][:], op=A.max)
                        nc.vector.tensor_tensor(f2[:], f1[:, 0:HALF // 2],
                                                f1[:, HALF // 2:], op=A.max)
                        nc.vector.tensor_tensor(f3[:], f2[:, 0:HALF // 4],
                                                f2[:, HALF // 4:], op=A.max)
                        nc.vector.tensor_reduce(rowacc[:, r: r + 1], f3[:],
                                                axis=X, op=A.max)

                # ---- batch finalize ----
                rs = acc.tile([128, 1], F32, name=f"rs_{it}")
                nc.vector.reduce_sum(rs[:], rowacc[:], axis=X)
                rsr = acc.tile([128, 1], F32, name=f"rsr_{it}")
                nc.gpsimd.partition_all_reduce(rsr[:], rs[:], channels=128,
                                               reduce_op=bass_isa.ReduceOp.add)
                if PAR_BF16:
                    nc.gpsimd.partition_all_reduce(cred[:], colrun[:], channels=128,
                                                   reduce_op=bass_isa.ReduceOp.max)
                    credv = cred
                else:
                    colf = acc.tile([128, N], F32, name=f"colf_{it}", tag="colf")
                    nc.vector.tensor_copy(colf[:], colrun[:])
                    credf = acc.tile([128, N], F32, name=f"credf_{it}", tag="credf")
                    nc.gpsimd.partition_all_reduce(credf[:], colf[:], channels=128,
                                                   reduce_op=bass_isa.ReduceOp.max)
                    credv = credf
                cs = acc.tile([1, 1], F32, name=f"cs_{it}")
                nc.vector.reduce_sum(cs[:], credv[0:1, :], axis=X)
                bt = acc.tile([1, 1], F32, name=f"bt_{it}")
                nc.vector.tensor_add(bt[:], rsr[0:1, 0:1], cs[:])
                if it == 0:
                    nc.vector.tensor_copy(stot[:], bt[:])
                else:
                    nc.vector.tensor_add(stot[:], stot[:], bt[:])

            nc.scalar.mul(s_out[:], stot[:], -1.0 / (N * repeat))
            nc.gpsimd.dma_start(out_d[:], s_out[:])
    nc.compile()
    return nc


def _build_operands(x, y):
    """x,y [B,N,3] f32 -> per-core input maps (augmented bf16 layouts)."""
    x = np.ascontiguousarray(x, np.float32)
    y = np.ascontiguousarray(y, np.float32)
    bf = ml_dtypes.bfloat16
    ones = np.ones(N, np.float32)
    in_maps = []
    for core in range(NCORES):
        ax_parts, ay_parts = [], []
        for b in range(core * BPC, (core + 1) * BPC):
            xb = x[b].astype(bf).astype(np.float32)      # [N,3] perturbed pts
            yb = y[b].astype(bf).astype(np.float32)
            x2 = (xb * xb).sum(1)
            y2 = (yb * yb).sum(1)
            xh = x2.astype(bf).astype(np.float32)
            xl = x2 - xh
            yh = y2.astype(bf).astype(np.float32)
            yl = y2 - yh
            ax_parts.append(np.stack(
                [2 * xb[:, 0], 2 * xb[:, 1], 2 * xb[:, 2], -xh, -xl, ones, ones], 0))
            ay_parts.append(np.stack(
                [yb[:, 0], yb[:, 1], yb[:, 2], ones, ones, -yh, -yl], 0))
        ax = np.concatenate(ax_parts, axis=1)            # [7, BPC*N]
        ay = np.concatenate(ay_parts, axis=1)
        in_maps.append({"ax": np.ascontiguousarray(ax.astype(bf)),
                        "ay": np.ascontiguousarray(ay.astype(bf))})
    return in_maps


_NC_CACHE = {}


def _get_nc(repeat: int = 1):
    if repeat not in _NC_CACHE:
        _NC_CACHE[repeat] = _build_nc(repeat)
    return _NC_CACHE[repeat]


def kernel(x, y):
    x = np.asarray(x, dtype=np.float32)
    y = np.asarray(y, dtype=np.float32)
    assert x.shape == (B, N, D3) and y.shape == (B, N, D3)
    in_maps = _build_operands(x, y)
    nc = _get_nc(1)
    res = run_bass_kernel_spmd(nc, in_maps, core_ids=list(range(NCORES)))
    total = sum(float(res.results[i]["out"][0, 0]) for i in range(NCORES))
    return np.float32(total / B)



# revision 2
# speedup vs baseline: 2.1864x; 2.1864x over previous
"""Chamfer distance kernel for Trainium2 (8 NeuronCores, data-parallel over batch).

Input : x, y float32 [16, 4096, 3]
Output: scalar float32 = mean_b [ mean_n min_m ||x_bn - y_bm||^2
                                + mean_m min_n ||x_bn - y_bm||^2 ]

This environment charges a large, roughly flat cost per *instruction*
(engines do not overlap), so the kernel minimizes instruction count by
computing squared distances directly on the Vector engine with giant
multi-dim access patterns (128 x 49152 elements per op, stride-0
broadcasts), instead of PE matmuls (which are capped at 512 columns
per instruction and would need 512+ instructions per core).

Per core (2 batches, 32 x-blocks of 128 each, groups of G=4 blocks):
  c[p,k,g,m] = (x_k[blk g,row p] - y_k[m])^2     1 TT sub + 1 TT mult
  u[p,g,m]   = c0+c1+c2                          2 TT adds (plane folds)
  rowacc     = min_m u                           1 segmented reduce
  colstack   = min over g-blocks                 2 TT folds
After 8 groups: fold colstack 8->1 (3 TT), DMA out per-batch col mins
[128, 4096] (f16) and rowacc [128, 64] (f32). Host does the final
128-partition min + sums (tiny numpy) -- cross-partition DVE inputs are
illegal on this target, and a DMA out is 1 instruction vs a 14-op tree.
"""
import sys

sys.path.insert(0, "/opt/trn_rl_repo")

import numpy as np

import concourse.bacc as bacc
import concourse.tile as tile
from concourse import mybir
from concourse.alu_op_type import AluOpType
from concourse.bass_utils import run_bass_kernel_spmd

F32 = mybir.dt.float32
F16 = mybir.dt.float16
X = mybir.AxisListType.X
A = AluOpType

B, N, KC = 16, 4096, 3
NCORES = 8
BPC = B // NCORES            # batches per core
NBLK = N // 128              # 32 x-blocks per batch
G = 4                        # blocks per group
NG = NBLK // G               # 8 groups per batch
GM = G * N                   # 16384, one k-plane per group
CW = KC * GM                 # 49152, full c tile width


def _build_nc(repeat: int = 1):
    nc = bacc.Bacc("TRN2", target_bir_lowering=False, debug=False, num_devices=NCORES)
    xp_d = nc.dram_tensor("xp", [128, BPC * NBLK * KC], F16, kind="ExternalInput").ap()
    yp_ds = [
        nc.dram_tensor(f"yp{b}", [128, N * KC], F16, kind="ExternalInput").ap()
        for b in range(BPC)
    ]
    col_d = nc.dram_tensor("col", [128, BPC * N], F16, kind="ExternalOutput").ap()
    row_d = nc.dram_tensor("row", [128, BPC * NBLK], F32, kind="ExternalOutput").ap()

    with tile.TileContext(nc) as tc:
        import contextlib
        with contextlib.ExitStack() as ctx:
            const = ctx.enter_context(tc.tile_pool(name="const", bufs=1))

            xp_t = const.tile([128, BPC * NBLK * KC], F16, name="xp_t")
            nc.gpsimd.dma_start(xp_t[:], xp_d[:])
            yp_t = const.tile([128, N * KC], F16, name="yp_t")
            c_t = const.tile([128, CW], F16, name="c_t")
            cs_t = const.tile([128, NG * N], F16, name="cs_t")
            rowacc = const.tile([128, BPC * NBLK], F32, name="rowacc")

            c4 = c_t[:].rearrange("p (k g m) -> p k g m", k=KC, g=G)
            y_ap = (
                yp_t[:]
                .rearrange("p (m k) -> p k m", k=KC)
                .unsqueeze(2)
                .broadcast_to([128, KC, G, N])
            )

            for _rep in range(repeat):
                for b in range(BPC):
                    nc.gpsimd.dma_start(yp_t[:], yp_ds[b][:])
                    for g in range(NG):
                        xs = xp_t[:, (b * NBLK + g * G) * KC:(b * NBLK + (g + 1) * G) * KC]
                        x_ap = (
                            xs.rearrange("p (g k) -> p k g", g=G)
                            .unsqueeze(3)
                            .broadcast_to([128, KC, G, N])
                        )
                        # c[p,k,g,m] = (x - y); then square in place
                        nc.vector.tensor_tensor(c4, x_ap, y_ap, op=A.subtract)
                        nc.vector.tensor_tensor(c_t[:], c_t[:], c_t[:], op=A.mult)
                        # u = c0 + c1 + c2 (into plane 0)
                        nc.vector.tensor_tensor(
                            c_t[:, 0:GM], c_t[:, 0:GM], c_t[:, GM:2 * GM], op=A.add)
                        nc.vector.tensor_tensor(
                            c_t[:, 0:GM], c_t[:, 0:GM], c_t[:, 2 * GM:3 * GM], op=A.add)
                        # row direction: min over m for each (p, g)
                        nc.vector.tensor_reduce(
                            rowacc[:, b * NBLK + g * G: b * NBLK + (g + 1) * G],
                            c_t[:, 0:GM].rearrange("p (g m) -> p g m", g=G),
                            axis=X, op=A.min)
                        # col direction: fold G blocks -> colstack slot g
                        nc.vector.tensor_tensor(
                            c_t[:, 0:2 * N], c_t[:, 0:2 * N], c_t[:, 2 * N:4 * N], op=A.min)
                        nc.vector.tensor_tensor(
                            cs_t[:, g * N:(g + 1) * N], c_t[:, 0:N], c_t[:, N:2 * N], op=A.min)
                    # fold colstack 8 -> 1
                    nc.vector.tensor_tensor(
                        cs_t[:, 0:4 * N], cs_t[:, 0:4 * N], cs_t[:, 4 * N:8 * N], op=A.min)
                    nc.vector.tensor_tensor(
                        cs_t[:, 0:2 * N], cs_t[:, 0:2 * N], cs_t[:, 2 * N:4 * N], op=A.min)
                    nc.vector.tensor_tensor(
                        cs_t[:, 0:N], cs_t[:, 0:N], cs_t[:, N:2 * N], op=A.min)
                    nc.gpsimd.dma_start(col_d[:, b * N:(b + 1) * N], cs_t[:, 0:N])

            nc.gpsimd.dma_start(row_d[:], rowacc[:])
    nc.compile()
    return nc


def _build_operands(x, y):
    """x,y [B,N,3] f32 -> per-core input maps (f16 packed layouts)."""
    x = np.asarray(x, np.float32).astype(np.float16)
    y = np.asarray(y, np.float32).astype(np.float16)
    in_maps = []
    for core in range(NCORES):
        xp = np.empty((128, BPC * NBLK * KC), np.float16)
        maps = {}
        for j in range(BPC):
            bg = core * BPC + j
            # xp[p, (j*NBLK + r)*3 + k] = x[bg, r*128 + p, k]
            xb = x[bg].reshape(NBLK, 128, KC).transpose(1, 0, 2).reshape(128, NBLK * KC)
            xp[:, j * NBLK * KC:(j + 1) * NBLK * KC] = xb
            # yp[p, m*3+k] = y[bg, m, k] replicated across partitions
            maps[f"yp{j}"] = np.ascontiguousarray(
                np.broadcast_to(y[bg].reshape(1, N * KC), (128, N * KC)))
        maps["xp"] = xp
        in_maps.append(maps)
    return in_maps


_NC_CACHE = {}


def _get_nc(repeat: int = 1):
    if repeat not in _NC_CACHE:
        _NC_CACHE[repeat] = _build_nc(repeat)
    return _NC_CACHE[repeat]


def _finalize(results):
    total = 0.0
    for core in range(NCORES):
        row = np.asarray(results[core]["row"], np.float32)   # [128, BPC*NBLK]
        col = np.asarray(results[core]["col"], np.float32)   # [128, BPC*N]
        for j in range(BPC):
            rsum = row[:, j * NBLK:(j + 1) * NBLK].sum(dtype=np.float64)
            csum = col[:, j * N:(j + 1) * N].min(axis=0).sum(dtype=np.float64)
            total += (rsum + csum) / N
    return np.float32(total / B)


def kernel(x, y):
    x = np.asarray(x, dtype=np.float32)
    y = np.asarray(y, dtype=np.float32)
    assert x.shape == (B, N, KC) and y.shape == (B, N, KC)
    in_maps = _build_operands(x, y)
    nc = _get_nc(1)
    res = run_bass_kernel_spmd(nc, in_maps, core_ids=list(range(NCORES)))
    return _finalize(res.results)


# revision 3
# speedup vs baseline: 9.5739x; 4.3789x over previous
"""Chamfer distance kernel for Trainium2 (8 NeuronCores, data-parallel over batch).

Input : x, y float32 [16, 4096, 3]
Output: scalar float32 = mean_b [ mean_n min_m ||x_bn - y_bm||^2
                                + mean_m min_n ||x_bn - y_bm||^2 ]

This environment charges a large, roughly flat cost per *instruction*
(engines do not overlap), so the kernel minimizes instruction count by
computing squared distances directly on the Vector engine with giant
multi-dim access patterns (128 x 49152 elements per op, stride-0
broadcasts), instead of PE matmuls (capped at 512 columns per
instruction, which would need 512+ instructions per core).

Per core (2 batches, 32 x-blocks of 128 rows each, groups of G=4 blocks):
  c[p,k,g,m] = x_k[blk g, row p] - y_k[m]      1 TT sub (4D broadcast AP)
  u = c0^2 + c1^2 + c2^2                       2 custom DVE ops (SQSQ, ADDSQ)
  rowacc[p, blk] = min_m u                     1 segmented reduce
  colstack[g] = min over the G blocks          2 TT folds
After 8 groups: fold colstack 8->1 (3 TT), DMA out per-batch column mins
[128, 4096] f16 and row mins [128, 64] f32. The host does the final
128-partition min + mean (tiny numpy) because cross-partition DVE inputs
are illegal on this target and a DMA out is 1 instruction vs a 14-op tree.
y coordinates are loaded via a partition-broadcast DMA (24 KB upload per
batch instead of 3 MB).
"""
import sys

sys.path.insert(0, "/opt/trn_rl_repo")

import numpy as np

import concourse.bacc as bacc
import concourse.tile as tile
from concourse import mybir
from concourse.alu_op_type import AluOpType
from concourse.bass_utils import run_bass_kernel_spmd

# --- custom DVE ops (registered at import time) ---------------------------
import concourse.dve_ops as dve_ops
from concourse.dve_ops import DveOp
from concourse.dve_spec import Spec, Src0, Src1, sq, lower, _has_src1


def _register_dve_op(name, spec):
    if name in dve_ops._SUB_OPCODE_FOR_NAME:
        for o in dve_ops.OPS:
            if o.name == name:
                return o
    row = dve_ops._CUSTOM_DVE_ROW_BASE + len(dve_ops.OPS)
    assert row < 0x20
    dve_ops._SUB_OPCODE_FOR_NAME[name] = row
    from concourse.dve_uop import DveOpSpec

    shas = {}
    for ver in ("v3", "v4"):
        try:
            uops = lower(spec, ver=ver)
            s = DveOpSpec(name=name, opcode=row, uops=uops, rd1_en=_has_src1(spec))
            shas[ver] = s.sha(ver)
        except Exception:
            pass
    op = DveOp(name, spec, subdim=False, uops_sha=shas)
    dve_ops.OPS.append(op)
    dve_ops.CUSTOM_DVE_SPECS[name] = spec
    return op


SQSQ = _register_dve_op(
    "SQSQ_ANT",
    Spec(
        body=sq(Src0) + sq(Src1),
        reference=lambda in0, in1, s0, s1, imm2: (
            in0.astype(np.float32) ** 2 + in1.astype(np.float32) ** 2
        ),
    ),
)
ADDSQ = _register_dve_op(
    "ADDSQ_ANT",
    Spec(
        body=Src0 + sq(Src1),
        reference=lambda in0, in1, s0, s1, imm2: (
            in0.astype(np.float32) + in1.astype(np.float32) ** 2
        ),
    ),
)
# ---------------------------------------------------------------------------

F32 = mybir.dt.float32
F16 = mybir.dt.float16
X = mybir.AxisListType.X
A = AluOpType

B, N, KC = 16, 4096, 3
NCORES = 8
BPC = B // NCORES            # batches per core
NBLK = N // 128              # 32 x-blocks per batch
G = 4                        # blocks per group
NG = NBLK // G               # 8 groups per batch
GM = G * N                   # 16384, one k-plane per group
CW = KC * GM                 # 49152, full c tile width


def _build_nc(repeat: int = 1):
    nc = bacc.Bacc("TRN2", target_bir_lowering=False, debug=False, num_devices=NCORES)
    xp_d = nc.dram_tensor("xp", [128, BPC * NBLK * KC], F16, kind="ExternalInput").ap()
    yp_ds = [
        nc.dram_tensor(f"yp{b}", [1, N * KC], F16, kind="ExternalInput").ap()
        for b in range(BPC)
    ]
    col_d = nc.dram_tensor("col", [128, BPC * N], F16, kind="ExternalOutput").ap()
    row_d = nc.dram_tensor("row", [128, BPC * NBLK], F32, kind="ExternalOutput").ap()

    with tile.TileContext(nc) as tc:
        import contextlib
        with contextlib.ExitStack() as ctx:
            const = ctx.enter_context(tc.tile_pool(name="const", bufs=1))

            xp_t = const.tile([128, BPC * NBLK * KC], F16, name="xp_t")
            nc.gpsimd.dma_start(xp_t[:], xp_d[:])
            yp_t = const.tile([128, N * KC], F16, name="yp_t")
            c_t = const.tile([128, CW], F16, name="c_t")
            cs_t = const.tile([128, NG * N], F16, name="cs_t")
            rowacc = const.tile([128, BPC * NBLK], F32, name="rowacc")

            c4 = c_t[:].rearrange("p (k g m) -> p k g m", k=KC, g=G)
            y_ap = (
                yp_t[:]
                .rearrange("p (m k) -> p k m", k=KC)
                .unsqueeze(2)
                .broadcast_to([128, KC, G, N])
            )

            for _rep in range(repeat):
                for b in range(BPC):
                    nc.gpsimd.dma_start(
                        yp_t[:], yp_ds[b][0:1, :].partition_broadcast(128).squeeze(1))
                    for g in range(NG):
                        xs = xp_t[:, (b * NBLK + g * G) * KC:(b * NBLK + (g + 1) * G) * KC]
                        x_ap = (
                            xs.rearrange("p (g k) -> p k g", g=G)
                            .unsqueeze(3)
                            .broadcast_to([128, KC, G, N])
                        )
                        # c[p,k,g,m] = x - y (3 diff planes)
                        nc.vector.tensor_tensor(c4, x_ap, y_ap, op=A.subtract)
                        # u = c0^2 + c1^2 + c2^2 into plane 0
                        nc.vector._custom_dve(
                            SQSQ, out=c_t[:, 0:GM],
                            in0=c_t[:, 0:GM], in1=c_t[:, GM:2 * GM])
                        nc.vector._custom_dve(
                            ADDSQ, out=c_t[:, 0:GM],
                            in0=c_t[:, 0:GM], in1=c_t[:, 2 * GM:3 * GM])
                        # row direction: min over m for each (p, g)
                        nc.vector.tensor_reduce(
                            rowacc[:, b * NBLK + g * G: b * NBLK + (g + 1) * G],
                            c_t[:, 0:GM].rearrange("p (g m) -> p g m", g=G),
                            axis=X, op=A.min)
                        # col direction: fold G blocks -> colstack slot g
                        nc.vector.tensor_tensor(
                            c_t[:, 0:2 * N], c_t[:, 0:2 * N], c_t[:, 2 * N:4 * N], op=A.min)
                        nc.vector.tensor_tensor(
                            cs_t[:, g * N:(g + 1) * N], c_t[:, 0:N], c_t[:, N:2 * N], op=A.min)
                    # fold colstack 8 -> 1
                    nc.vector.tensor_tensor(
                        cs_t[:, 0:4 * N], cs_t[:, 0:4 * N], cs_t[:, 4 * N:8 * N], op=A.min)
                    nc.vector.tensor_tensor(
                        cs_t[:, 0:2 * N], cs_t[:, 0:2 * N], cs_t[:, 2 * N:4 * N], op=A.min)
                    nc.vector.tensor_tensor(
                        cs_t[:, 0:N], cs_t[:, 0:N], cs_t[:, N:2 * N], op=A.min)
                    nc.gpsimd.dma_start(col_d[:, b * N:(b + 1) * N], cs_t[:, 0:N])

            nc.gpsimd.dma_start(row_d[:], rowacc[:])
    nc.compile()
    return nc


def _build_operands(x, y):
    """x,y [B,N,3] f32 -> per-core input maps (f16 packed layouts)."""
    x = np.asarray(x, np.float32).astype(np.float16)
    y = np.asarray(y, np.float32).astype(np.float16)
    in_maps = []
    for core in range(NCORES):
        xp = np.empty((128, BPC * NBLK * KC), np.float16)
        maps = {}
        for j in range(BPC):
            bg = core * BPC + j
            # xp[p, (j*NBLK + r)*3 + k] = x[bg, r*128 + p, k]
            xb = x[bg].reshape(NBLK, 128, KC).transpose(1, 0, 2).reshape(128, NBLK * KC)
            xp[:, j * NBLK * KC:(j + 1) * NBLK * KC] = xb
            # yp[m*3+k] = y[bg, m, k]; broadcast to 128 partitions on-device
            maps[f"yp{j}"] = np.ascontiguousarray(y[bg].reshape(1, N * KC))
        maps["xp"] = xp
        in_maps.append(maps)
    return in_maps


_NC_CACHE = {}


def _get_nc(repeat: int = 1):
    if repeat not in _NC_CACHE:
        _NC_CACHE[repeat] = _build_nc(repeat)
    return _NC_CACHE[repeat]


def _finalize(results):
    total = 0.0
    for core in range(NCORES):
        row = np.asarray(results[core]["row"], np.float32)   # [128, BPC*NBLK]
        col = np.asarray(results[core]["col"], np.float32)   # [128, BPC*N]
        for j in range(BPC):
            rsum = row[:, j * NBLK:(j + 1) * NBLK].sum(dtype=np.float64)
            csum = col[:, j * N:(j + 1) * N].min(axis=0).sum(dtype=np.float64)
            total += (rsum + csum) / N
    return np.float32(total / B)


def kernel(x, y):
    x = np.asarray(x, dtype=np.float32)
    y = np.asarray(y, dtype=np.float32)
    assert x.shape == (B, N, KC) and y.shape == (B, N, KC)
    in_maps = _build_operands(x, y)
    nc = _get_nc(1)
    res = run_bass_kernel_spmd(nc, in_maps, core_ids=list(range(NCORES)))
    return _finalize(res.results)


# revision 5
# speedup vs baseline: 12.1754x; 1.2717x over previous
"""Chamfer distance kernel for Trainium2 (8 NeuronCores, data-parallel over batch).

Input : x, y float32 [16, 4096, 3]
Output: scalar float32 = mean_b [ mean_n min_m ||x_bn - y_bm||^2
                                + mean_m min_n ||x_bn - y_bm||^2 ]

This environment charges a large, roughly flat cost per *instruction*
(engines do not overlap), so the kernel minimizes instruction count by
computing squared distances directly on the Vector engine with giant
multi-dim access patterns (128 x 49152 elements per op, stride-0
broadcasts), instead of PE matmuls (capped at 512 columns per
instruction, which would need 512+ instructions per core).

Per core (2 batches, 32 x-blocks of 128 rows each, groups of G=4 blocks):
  c[p,k,g,m] = x_k[blk g, row p] - y_k[m]      1 TT sub (4D broadcast AP)
  u = c0^2 + c1^2 + c2^2                       2 custom DVE ops (SQSQ, ADDSQ)
  rowacc[p, blk] = min_m u                     1 segmented reduce
  colstack[g] = min over the G blocks          2 TT folds
After 8 groups: fold colstack 8->1 (3 TT), DMA out per-batch column mins
[128, 4096] f16 and row mins [128, 64] f32. The host does the final
128-partition min + mean (tiny numpy) because cross-partition DVE inputs
are illegal on this target and a DMA out is 1 instruction vs a 14-op tree.
y coordinates are loaded via a partition-broadcast DMA (24 KB upload per
batch instead of 3 MB).
"""
import sys

sys.path.insert(0, "/opt/trn_rl_repo")

import numpy as np

import concourse.bacc as bacc
import concourse.tile as tile
from concourse import mybir
from concourse.alu_op_type import AluOpType
from concourse.bass_utils import run_bass_kernel_spmd

# --- custom DVE ops (registered at import time) ---------------------------
import concourse.dve_ops as dve_ops
from concourse.dve_ops import DveOp
from concourse.dve_spec import Spec, Src0, Src1, sq, lower, _has_src1


def _register_dve_op(name, spec):
    if name in dve_ops._SUB_OPCODE_FOR_NAME:
        for o in dve_ops.OPS:
            if o.name == name:
                return o
    row = dve_ops._CUSTOM_DVE_ROW_BASE + len(dve_ops.OPS)
    assert row < 0x20
    dve_ops._SUB_OPCODE_FOR_NAME[name] = row
    from concourse.dve_uop import DveOpSpec

    shas = {}
    for ver in ("v3", "v4"):
        try:
            uops = lower(spec, ver=ver)
            s = DveOpSpec(name=name, opcode=row, uops=uops, rd1_en=_has_src1(spec))
            shas[ver] = s.sha(ver)
        except Exception:
            pass
    op = DveOp(name, spec, subdim=False, uops_sha=shas)
    dve_ops.OPS.append(op)
    dve_ops.CUSTOM_DVE_SPECS[name] = spec
    return op


SQSQ = _register_dve_op(
    "SQSQ_ANT",
    Spec(
        body=sq(Src0) + sq(Src1),
        reference=lambda in0, in1, s0, s1, imm2: (
            in0.astype(np.float32) ** 2 + in1.astype(np.float32) ** 2
        ),
    ),
)
ADDSQ = _register_dve_op(
    "ADDSQ_ANT",
    Spec(
        body=Src0 + sq(Src1),
        reference=lambda in0, in1, s0, s1, imm2: (
            in0.astype(np.float32) + in1.astype(np.float32) ** 2
        ),
    ),
)
# ---------------------------------------------------------------------------

F32 = mybir.dt.float32
F16 = mybir.dt.float16
X = mybir.AxisListType.X
A = AluOpType

B, N, KC = 16, 4096, 3
NCORES = 8
BPC = B // NCORES            # batches per core
NBLK = N // 128              # 32 x-blocks per batch
G = 4                        # blocks per group
NG = NBLK // G               # 8 groups per batch
GM = G * N                   # 16384, one k-plane per group
CW = KC * GM                 # 49152, full c tile width


def _build_nc(repeat: int = 1):
    nc = bacc.Bacc("TRN2", target_bir_lowering=False, debug=False, num_devices=NCORES)
    xp_d = nc.dram_tensor("xp", [128, BPC * NBLK * KC], F16, kind="ExternalInput").ap()
    yp_ds = [
        nc.dram_tensor(f"yp{b}", [1, N * KC], F16, kind="ExternalInput").ap()
        for b in range(BPC)
    ]
    col_d = nc.dram_tensor("col", [128, BPC * N], F16, kind="ExternalOutput").ap()
    row_d = nc.dram_tensor("row", [128, BPC * NBLK], F32, kind="ExternalOutput").ap()

    with tile.TileContext(nc) as tc:
        import contextlib
        with contextlib.ExitStack() as ctx:
            const = ctx.enter_context(tc.tile_pool(name="const", bufs=1))

            xp_t = const.tile([128, BPC * NBLK * KC], F16, name="xp_t")
            nc.gpsimd.dma_start(xp_t[:], xp_d[:])
            yp_t = const.tile([128, N * KC], F16, name="yp_t")
            c_t = const.tile([128, CW], F16, name="c_t")
            cs_t = const.tile([128, NG * N], F16, name="cs_t")
            colout = const.tile([128, BPC * N], F16, name="colout")
            rowacc = const.tile([128, BPC * NBLK], F32, name="rowacc")

            c4 = c_t[:].rearrange("p (k g m) -> p k g m", k=KC, g=G)
            y_ap = (
                yp_t[:]
                .rearrange("p (m k) -> p k m", k=KC)
                .unsqueeze(2)
                .broadcast_to([128, KC, G, N])
            )

            for _rep in range(repeat):
                for b in range(BPC):
                    nc.gpsimd.dma_start(
                        yp_t[:], yp_ds[b][0:1, :].partition_broadcast(128).squeeze(1))
                    for g in range(NG):
                        xs = xp_t[:, (b * NBLK + g * G) * KC:(b * NBLK + (g + 1) * G) * KC]
                        x_ap = (
                            xs.rearrange("p (g k) -> p k g", g=G)
                            .unsqueeze(3)
                            .broadcast_to([128, KC, G, N])
                        )
                        # c[p,k,g,m] = x - y (3 diff planes)
                        nc.vector.tensor_tensor(c4, x_ap, y_ap, op=A.subtract)
                        # u = c0^2 + c1^2 + c2^2 into plane 0
                        nc.vector._custom_dve(
                            SQSQ, out=c_t[:, 0:GM],
                            in0=c_t[:, 0:GM], in1=c_t[:, GM:2 * GM])
                        nc.vector._custom_dve(
                            ADDSQ, out=c_t[:, 0:GM],
                            in0=c_t[:, 0:GM], in1=c_t[:, 2 * GM:3 * GM])
                        # row direction: min over m for each (p, g)
                        nc.vector.tensor_reduce(
                            rowacc[:, b * NBLK + g * G: b * NBLK + (g + 1) * G],
                            c_t[:, 0:GM].rearrange("p (g m) -> p g m", g=G),
                            axis=X, op=A.min)
                        # col direction: min over the G blocks -> colstack slot g
                        nc.vector.tensor_reduce(
                            cs_t[:, g * N:(g + 1) * N],
                            c_t[:, 0:GM].rearrange("p (g m) -> p m g", g=G),
                            axis=X, op=A.min)
                    # min over the NG group slots -> per-batch column mins
                    nc.vector.tensor_reduce(
                        colout[:, b * N:(b + 1) * N],
                        cs_t[:].rearrange("p (g m) -> p m g", g=NG),
                        axis=X, op=A.min)

            nc.gpsimd.dma_start(col_d[:], colout[:])
            nc.gpsimd.dma_start(row_d[:], rowacc[:])
    nc.compile()
    return nc


def _build_operands(x, y):
    """x,y [B,N,3] f32 -> per-core input maps (f16 packed layouts)."""
    x = np.asarray(x, np.float32).astype(np.float16)
    y = np.asarray(y, np.float32).astype(np.float16)
    in_maps = []
    for core in range(NCORES):
        xp = np.empty((128, BPC * NBLK * KC), np.float16)
        maps = {}
        for j in range(BPC):
            bg = core * BPC + j
            # xp[p, (j*NBLK + r)*3 + k] = x[bg, r*128 + p, k]
            xb = x[bg].reshape(NBLK, 128, KC).transpose(1, 0, 2).reshape(128, NBLK * KC)
            xp[:, j * NBLK * KC:(j + 1) * NBLK * KC] = xb
            # yp[m*3+k] = y[bg, m, k]; broadcast to 128 partitions on-device
            maps[f"yp{j}"] = np.ascontiguousarray(y[bg].reshape(1, N * KC))
        maps["xp"] = xp
        in_maps.append(maps)
    return in_maps


_NC_CACHE = {}


def _get_nc(repeat: int = 1):
    if repeat not in _NC_CACHE:
        _NC_CACHE[repeat] = _build_nc(repeat)
    return _NC_CACHE[repeat]


def _finalize(results):
    total = 0.0
    for core in range(NCORES):
        row = np.asarray(results[core]["row"], np.float32)   # [128, BPC*NBLK]
        col = np.asarray(results[core]["col"], np.float32)   # [128, BPC*N]
        for j in range(BPC):
            rsum = row[:, j * NBLK:(j + 1) * NBLK].sum(dtype=np.float64)
            csum = col[:, j * N:(j + 1) * N].min(axis=0).sum(dtype=np.float64)
            total += (rsum + csum) / N
    return np.float32(total / B)


def kernel(x, y):
    x = np.asarray(x, dtype=np.float32)
    y = np.asarray(y, dtype=np.float32)
    assert x.shape == (B, N, KC) and y.shape == (B, N, KC)
    in_maps = _build_operands(x, y)
    nc = _get_nc(1)
    res = run_bass_kernel_spmd(nc, in_maps, core_ids=list(range(NCORES)))
    return _finalize(res.results)


# revision 16
# speedup vs baseline: 14.7752x; 1.2135x over previous
"""Chamfer distance kernel for Trainium2 (8 NeuronCores, data-parallel over batch).

Input : x, y float32 [16, 4096, 3]
Output: scalar float32 = mean_b [ mean_n min_m ||x_bn - y_bm||^2
                                + mean_m min_n ||x_bn - y_bm||^2 ]

This environment charges a large, roughly flat cost per *instruction*
(engines do not overlap), so the kernel minimizes instruction count by
computing squared distances directly on the Vector engine with giant
multi-dim access patterns (128 x 49152 elements per op, stride-0
broadcasts), instead of PE matmuls (capped at 512 columns per
instruction, which would need 512+ instructions per core).

Per core (2 batches, 32 x-blocks of 128 rows each, groups of G=4 blocks):
  c[p,k,g,m] = x_k[blk g, row p] - y_k[m]      1 TT sub (4D broadcast AP)
  u = c0^2 + c1^2 + c2^2                       2 custom DVE ops (SQSQ, ADDSQ)
  rowacc[p, blk] = min_m u                     1 segmented reduce
  colstack[g] = min over the G blocks          2 TT folds
After 8 groups: fold colstack 8->1 (3 TT), DMA out per-batch column mins
[128, 4096] f16 and row mins [128, 64] f32. The host does the final
128-partition min + mean (tiny numpy) because cross-partition DVE inputs
are illegal on this target and a DMA out is 1 instruction vs a 14-op tree.
y coordinates are loaded via a partition-broadcast DMA (24 KB upload per
batch instead of 3 MB).
"""
import sys

sys.path.insert(0, "/opt/trn_rl_repo")

import numpy as np

import concourse.bacc as bacc
import concourse.tile as tile
from concourse import mybir
from concourse.alu_op_type import AluOpType
from concourse.bass_utils import run_bass_kernel_spmd

# --- custom DVE ops (registered at import time) ---------------------------
import concourse.dve_ops as dve_ops
from concourse.dve_ops import DveOp
from concourse.dve_spec import Spec, Src0, Src1, sq, lower, _has_src1


def _register_dve_op(name, spec):
    if name in dve_ops._SUB_OPCODE_FOR_NAME:
        for o in dve_ops.OPS:
            if o.name == name:
                return o
    row = dve_ops._CUSTOM_DVE_ROW_BASE + len(dve_ops.OPS)
    assert row < 0x20
    dve_ops._SUB_OPCODE_FOR_NAME[name] = row
    from concourse.dve_uop import DveOpSpec

    shas = {}
    for ver in ("v3", "v4"):
        try:
            uops = lower(spec, ver=ver)
            s = DveOpSpec(name=name, opcode=row, uops=uops, rd1_en=_has_src1(spec))
            shas[ver] = s.sha(ver)
        except Exception:
            pass
    op = DveOp(name, spec, subdim=False, uops_sha=shas)
    dve_ops.OPS.append(op)
    dve_ops.CUSTOM_DVE_SPECS[name] = spec
    return op


SQSQ = _register_dve_op(
    "SQSQ_ANT",
    Spec(
        body=sq(Src0) + sq(Src1),
        reference=lambda in0, in1, s0, s1, imm2: (
            in0.astype(np.float32) ** 2 + in1.astype(np.float32) ** 2
        ),
    ),
)
ADDSQ = _register_dve_op(
    "ADDSQ_ANT",
    Spec(
        body=Src0 + sq(Src1),
        reference=lambda in0, in1, s0, s1, imm2: (
            in0.astype(np.float32) + in1.astype(np.float32) ** 2
        ),
    ),
)
# ---------------------------------------------------------------------------

F32 = mybir.dt.float32
F16 = mybir.dt.float16
X = mybir.AxisListType.X
A = AluOpType

B, N, KC = 16, 4096, 3
NCORES = 8
BPC = B // NCORES            # batches per core
NBLK = N // 128              # 32 x-blocks per batch
G = 8                        # blocks per group
NG = NBLK // G               # 4 groups per batch
GM = G * N                   # 32768 elements per k-plane
CW = 2 * GM                  # c holds two planes, interleaved at g granularity:
                             # c[p, g*2N + k*N + m], so every AP stride <= 8192
                             # (ISA step_elem and num_elem fields are 16-bit)


def _build_nc(repeat: int = 1):
    nc = bacc.Bacc("TRN2", target_bir_lowering=False, debug=False, num_devices=NCORES)
    xp_d = nc.dram_tensor("xp", [128, BPC * NBLK * KC], F16, kind="ExternalInput").ap()
    yp_ds = [
        nc.dram_tensor(f"yp{b}", [1, N * KC], F16, kind="ExternalInput").ap()
        for b in range(BPC)
    ]
    col_d = nc.dram_tensor("col", [128, BPC * N], F16, kind="ExternalOutput").ap()
    row_d = nc.dram_tensor("row", [128, BPC * NBLK], F32, kind="ExternalOutput").ap()

    with tile.TileContext(nc) as tc:
        import contextlib
        with contextlib.ExitStack() as ctx:
            const = ctx.enter_context(tc.tile_pool(name="const", bufs=1))

            xp_t = const.tile([128, BPC * NBLK * KC], F16, name="xp_t")
            nc.gpsimd.dma_start(xp_t[:], xp_d[:])
            yp_t = const.tile([128, N * KC], F16, name="yp_t")
            c_t = const.tile([128, CW], F16, name="c_t")
            cs_t = const.tile([128, NG * N], F16, name="cs_t")
            colout = const.tile([128, BPC * N], F16, name="colout")
            rowacc = const.tile([128, BPC * NBLK], F32, name="rowacc")

            # c layout [p, g, k, m]; sub iterates [p, k, g, m]
            c4 = c_t[:].rearrange("p (g k m) -> p k g m", g=G, k=2)
            u_v = c4[:, 0, :, :]                      # [128, G, N] k=0 subplanes
            v_v = c4[:, 1, :, :]                      # [128, G, N] k=1 subplanes
            cB = c4[:, 1:2, :, :]
            ykm = yp_t[:].rearrange("p (m k) -> p k m", k=KC)
            y_apA = ykm[:, 0:2, :].unsqueeze(2).broadcast_to([128, 2, G, N])
            y_apB = ykm[:, 2:3, :].unsqueeze(2).broadcast_to([128, 1, G, N])

            for _rep in range(repeat):
                for b in range(BPC):
                    nc.gpsimd.dma_start(
                        yp_t[:], yp_ds[b][0:1, :].partition_broadcast(128).squeeze(1))
                    for g in range(NG):
                        xs = xp_t[:, (b * NBLK + g * G) * KC:(b * NBLK + (g + 1) * G) * KC]
                        xkg = xs.rearrange("p (g k) -> p k g", g=G)
                        x_apA = xkg[:, 0:2, :].unsqueeze(3).broadcast_to([128, 2, G, N])
                        x_apB = xkg[:, 2:3, :].unsqueeze(3).broadcast_to([128, 1, G, N])
                        # subplanes 0,1 = (x0-y0), (x1-y1)
                        nc.vector.tensor_tensor(c4, x_apA, y_apA, op=A.subtract)
                        # subplane0 = d0^2 + d1^2
                        nc.vector._custom_dve(SQSQ, out=u_v, in0=u_v, in1=v_v)
                        # subplane1 = (x2-y2)
                        nc.vector.tensor_tensor(cB, x_apB, y_apB, op=A.subtract)
                        # subplane0 += subplane1^2  -> u
                        nc.vector._custom_dve(ADDSQ, out=u_v, in0=u_v, in1=v_v)
                        # row direction: min over m for each (p, g)
                        nc.vector.tensor_reduce(
                            rowacc[:, b * NBLK + g * G: b * NBLK + (g + 1) * G],
                            u_v, axis=X, op=A.min)
                        # col direction: min over the G blocks -> colstack slot g
                        nc.vector.tensor_reduce(
                            cs_t[:, g * N:(g + 1) * N],
                            c_t[:].rearrange("p (g k m) -> p k m g", g=G, k=2)[:, 0, :, :],
                            axis=X, op=A.min)
                    # min over the NG group slots -> per-batch column mins
                    nc.vector.tensor_reduce(
                        colout[:, b * N:(b + 1) * N],
                        cs_t[:].rearrange("p (g m) -> p m g", g=NG),
                        axis=X, op=A.min)

            nc.gpsimd.dma_start(col_d[:], colout[:])
            nc.gpsimd.dma_start(row_d[:], rowacc[:])
    nc.compile()
    return nc


def _build_operands(x, y):
    """x,y [B,N,3] f32 -> per-core input maps (f16 packed layouts)."""
    x = np.asarray(x, np.float32).astype(np.float16)
    y = np.asarray(y, np.float32).astype(np.float16)
    in_maps = []
    for core in range(NCORES):
        xp = np.empty((128, BPC * NBLK * KC), np.float16)
        maps = {}
        for j in range(BPC):
            bg = core * BPC + j
            # xp[p, (j*NBLK + r)*3 + k] = x[bg, r*128 + p, k]
            xb = x[bg].reshape(NBLK, 128, KC).transpose(1, 0, 2).reshape(128, NBLK * KC)
            xp[:, j * NBLK * KC:(j + 1) * NBLK * KC] = xb
            # yp[m*3+k] = y[bg, m, k]; broadcast to 128 partitions on-device
            maps[f"yp{j}"] = np.ascontiguousarray(y[bg].reshape(1, N * KC))
        maps["xp"] = xp
        in_maps.append(maps)
    return in_maps


_NC_CACHE = {}


def _get_nc(repeat: int = 1):
    if repeat not in _NC_CACHE:
        _NC_CACHE[repeat] = _build_nc(repeat)
    return _NC_CACHE[repeat]


def _finalize(results):
    total = 0.0
    for core in range(NCORES):
        row = np.asarray(results[core]["row"], np.float32)   # [128, BPC*NBLK]
        col = np.asarray(results[core]["col"], np.float32)   # [128, BPC*N]
        for j in range(BPC):
            rsum = row[:, j * NBLK:(j + 1) * NBLK].sum(dtype=np.float64)
            csum = col[:, j * N:(j + 1) * N].min(axis=0).sum(dtype=np.float64)
            total += (rsum + csum) / N
    return np.float32(total / B)


def kernel(x, y):
    x = np.asarray(x, dtype=np.float32)
    y = np.asarray(y, dtype=np.float32)
    assert x.shape == (B, N, KC) and y.shape == (B, N, KC)
    in_maps = _build_operands(x, y)
    nc = _get_nc(1)
    res = run_bass_kernel_spmd(nc, in_maps, core_ids=list(range(NCORES)))
    return _finalize(res.results)
